# revision 4
# baseline (speedup 1.0000x reference)
import sys
for _p in ("/opt/trn_rl_repo",):
    if _p not in sys.path:
        sys.path.insert(0, _p)
"""GAT 2-layer kernel for TRN2, 8-core dst-sharded, bf16 tables.

v2 design:
- bf16 gather tables (768B rows layer0, 256B rows layer1)
- no AllGather for layer 0: each core computes the full [h|al_src] table
  redundantly on PE (cheaper than the collective)
- layer-1 table AllGathered (optionally chunked); its region rows are
  permuted [chunk][core][pos] to match AllGather concatenation order
- W2 one-hot (edge-partition) and W2T one-hot (dst-partition) both built
  on DVE from contiguous tensor_scalar is_equal ops; the dst-offsets are
  replicated across partitions with a K=1 ones matmul on PE
- ACT runs Exp only inside the loops (log-softmax Ln batched at the end)
- ELU's "-1" folded into the layer-1 bias row host-side
"""

import numpy as np
from contextlib import ExitStack

import concourse.bass as bass
import concourse.bacc as bacc
import concourse.mybir as mybir
import concourse.tile as tile

dt = mybir.dt
F32 = dt.float32
BF16 = dt.bfloat16
AL = mybir.AluOpType
ACT = mybir.ActivationFunctionType
NPBF16 = mybir.dt.np(BF16)

NEG_SLOPE = 0.2


# ----------------------------------------------------------------- host prep

def make_cfg(N, E, F_in, H, D, OUT, cores, split=32000, ag_chunks=1):
    assert N % cores == 0
    own = N // cores
    nw = (own + 127) // 128
    cfg = dict(
        N=N, E=E, F_IN=F_in, H=H, D=D, OUT=OUT, CORES=cores, OWN=own,
        WIN=128, NW=nw,
        AGC=min(ag_chunks, nw),
        SPLIT=min(split, N),
        ROW0=384,                       # bf16 elems (768B): h(256)|al_s(8)|pad
        ROW1=128,                       # bf16 elems (256B): t1(64)|as1(1)|pad
        NROWS=N + 2,
        SENT_A=0,
    )
    cfg["SENT_B"] = N + 1 - (cfg["SPLIT"] + 1)
    assert cfg["SPLIT"] + 1 <= 32767 and cfg["SENT_B"] <= 32767
    wruns = np.array_split(np.arange(nw), cfg["AGC"])
    chunks = []
    for run in wruns:
        w0, w1 = int(run[0]), int(run[-1]) + 1
        chunks.append((w0, w1, w0 * 128, min(w1 * 128, own)))
    cfg["CHUNKS"] = chunks
    return cfg


def wrap_idx(iv):
    """[n*16] int -> [128, n] int16 gather layout (t -> (t%16, t//16))."""
    iv = np.asarray(iv, np.int64)
    assert len(iv) % 16 == 0
    cols = len(iv) // 16
    a = iv.reshape(cols, 16).T.astype(np.int16)
    return np.tile(a, (8, 1))


def _bucket(cfg, src_row0, dst, owner, winid):
    """Per-layer edge bucketing. src_row0: 0-based region row of each edge's
    source node (region row = src_row0 + 1; rows 0 / N+1 are sentinels)."""
    N, CORES, OWN, WIN, NW = (cfg[k] for k in ("N", "CORES", "OWN", "WIN", "NW"))
    SPLIT = cfg["SPLIT"]
    hi = (src_row0 >= SPLIT).astype(np.int64)
    counts = np.zeros((CORES, NW, 2), np.int64)
    np.add.at(counts, (owner, winid, hi), 1)
    lo_slots = np.maximum(128, np.ceil(counts[:, :, 0].max(0) / 128).astype(np.int64) * 128)
    hi_slots = np.maximum(128, np.ceil(counts[:, :, 1].max(0) / 128).astype(np.int64) * 128)
    tpw = (lo_slots + hi_slots) // 128
    t_tiles = int(tpw.sum())
    ntp = int(tpw.max())
    idx_cols = int((lo_slots + hi_slots).sum() // 16)

    order = np.lexsort((hi, winid, owner))
    src_s = src_row0[order]
    dst_s = dst[order]
    own_s = owner[order]

    idx_all, dstoh_all, dstohT_all, dstfP_all = [], [], [], []
    for c in range(CORES):
        idx_list = []
        dstc = np.full((128, t_tiles), -1.0, np.float32)
        c0 = np.searchsorted(own_s, c)
        c1 = np.searchsorted(own_s, c + 1)
        cs, cd = src_s[c0:c1], dst_s[c0:c1]
        pos = 0
        tile_base = 0
        for w in range(NW):
            n_lo = int(counts[c, w, 0]); n_hi = int(counts[c, w, 1])
            L = int(lo_slots[w]); Hs = int(hi_slots[w])
            s_lo = cs[pos:pos + n_lo]; d_lo = cd[pos:pos + n_lo]
            s_hi = cs[pos + n_lo:pos + n_lo + n_hi]
            d_hi = cd[pos + n_lo:pos + n_lo + n_hi]
            pos += n_lo + n_hi
            idxA = np.full(L, cfg["SENT_A"], np.int64)
            idxA[:n_lo] = s_lo + 1
            idxB = np.full(Hs, cfg["SENT_B"], np.int64)
            idxB[:n_hi] = s_hi - SPLIT
            idx_list.append(wrap_idx(idxA))
            idx_list.append(wrap_idx(idxB))
            dl = np.full(L + Hs, -1.0, np.float32)
            dl[:n_lo] = (d_lo - c * OWN - w * WIN).astype(np.float32)
            dl[L:L + n_hi] = (d_hi - c * OWN - w * WIN).astype(np.float32)
            nt = (L + Hs) // 128
            dstc[:, tile_base:tile_base + nt] = dl.reshape(nt, 128).T
            tile_base += nt
        idx_arr = np.concatenate(idx_list, axis=1)
        assert idx_arr.shape == (128, idx_cols)
        idx_all.append(idx_arr)
        oh = (dstc[:, :, None] == np.arange(128, dtype=np.float32)).astype(NPBF16)
        dstoh_all.append(np.ascontiguousarray(oh.reshape(128, t_tiles * 128)))
        ohT = np.ascontiguousarray(np.transpose(oh, (2, 1, 0)).reshape(128, t_tiles * 128))
        dstohT_all.append(ohT)
        dstfP_all.append(np.ascontiguousarray(dstc.T.reshape(1, -1)).astype(NPBF16))
    return dict(idx=idx_all, dstoh=dstoh_all, dstohT=dstohT_all, dstf=dstfP_all,
                lo_slots=lo_slots.tolist(), hi_slots=hi_slots.tolist(),
                tpw=tpw.tolist(), t_tiles=t_tiles, ntp=ntp, idx_cols=idx_cols)


def prepare(x, edge_index, weights, cfg):
    N, E, CORES, OWN, WIN, NW = (cfg[k] for k in
                                 ("N", "E", "CORES", "OWN", "WIN", "NW"))
    H, D, F_IN, OUT = cfg["H"], cfg["D"], cfg["F_IN"], cfg["OUT"]
    chunks = cfg["CHUNKS"]

    src = np.concatenate([np.asarray(edge_index[0]), np.arange(N)]).astype(np.int64)
    dst = np.concatenate([np.asarray(edge_index[1]), np.arange(N)]).astype(np.int64)
    owner = dst // OWN
    winid = (dst - owner * OWN) // WIN

    ck_w0 = np.asarray([c[0] for c in chunks], np.int64)
    ck_r0 = np.asarray([c[2] for c in chunks], np.int64)
    ck_sz = np.asarray([c[3] - c[2] for c in chunks], np.int64)
    sc = src // OWN
    sp = src - sc * OWN
    sw = sp // WIN
    sk = np.searchsorted(ck_w0, sw, side="right") - 1
    src_r1 = CORES * ck_r0[sk] + sc * ck_sz[sk] + (sp - ck_r0[sk])

    b0 = _bucket(cfg, src, dst, owner, winid)
    b1 = _bucket(cfg, src_r1, dst, owner, winid)

    W0 = weights["W0"]; a_s0 = weights["a_src0"]; a_d0 = weights["a_dst0"]
    Wr0 = weights["Wr0"]; W1 = weights["W1"]; a_s1 = weights["a_src1"]
    a_d1 = weights["a_dst1"]; Wr1 = weights["Wr1"]
    blk0s = np.zeros((256, H), np.float32)
    blk0d = np.zeros((256, H), np.float32)
    for k in range(H):
        blk0s[k * D:(k + 1) * D, k] = a_s0[k]
        blk0d[k * D:(k + 1) * D, k] = a_d0[k]
    w_as0 = W0 @ blk0s
    w_ad0 = W0 @ blk0d
    wcat0 = np.concatenate([W0, w_as0], axis=1)              # [256, 264]
    w_as1 = W1 @ a_s1.reshape(-1, 1)
    w_ad1 = W1 @ a_d1.reshape(-1, 1)
    wcat1 = np.concatenate([W1, w_as1, w_ad1, Wr1], axis=1)  # [256, 130]
    bias0 = (weights["b0"] + weights["br0"]).astype(np.float32)
    bias1 = (weights["b1"] + weights["br1"]).astype(np.float32)
    biasr1 = np.zeros((1, wcat1.shape[1]), np.float32)
    biasr1[0, OUT + 2:] = bias1
    # ELU stores h1+1 (the -1 is folded into this bias row):
    # (h1+1)@wcat1 + (biasr1 - colsum(wcat1)) == h1@wcat1 + biasr1
    wcat1b = wcat1.astype(NPBF16)
    biasr1_adj = biasr1 - wcat1b.astype(np.float32).sum(axis=0, keepdims=True)

    xT = np.ascontiguousarray(x.T).astype(NPBF16)            # [256, N]
    consts = dict(
        xTb=xT,
        wcat0b=wcat0.astype(NPBF16),
        wad0b=w_ad0.astype(NPBF16),
        wr0b=Wr0.astype(NPBF16),
        wcat1b=wcat1b,
        biasr1b=biasr1_adj.astype(NPBF16),
        bias0=np.ascontiguousarray(bias0.reshape(2, 128).T),  # [128, KCH]
        identf=np.eye(128, dtype=np.float32),
        iotab=np.tile(np.arange(128).astype(NPBF16), (128, 1)),
        identb=np.eye(128, dtype=NPBF16),
        iotaPf=np.arange(128, dtype=np.float32).reshape(128, 1),
        onesb=np.ones((1, 128), NPBF16),
    )
    in_maps = []
    for c in range(CORES):
        m = dict(idx0=b0["idx"][c], dstoh0=b0["dstoh"][c], dstf0=b0["dstf"][c],
                 dstohT0=b0["dstohT"][c], dstohT1=b1["dstohT"][c],
                 idx1=b1["idx"][c], dstoh1=b1["dstoh"][c], dstf1=b1["dstf"][c],
                 xTo=np.ascontiguousarray(xT[:, c * OWN:(c + 1) * OWN]))
        m.update(consts)
        in_maps.append(m)
    meta = dict(
        l0={k: b0[k] for k in ("lo_slots", "hi_slots", "tpw", "t_tiles",
                               "ntp", "idx_cols")},
        l1={k: b1[k] for k in ("lo_slots", "hi_slots", "tpw", "t_tiles",
                               "ntp", "idx_cols")},
    )
    return in_maps, meta


# --------------------------------------------------------------- bass builder

def build(cfg, meta, repeat=1, sub=4, gbufs=3, vbufs=3, wbufs=3,
          w2t_host=True, sim_no_collective=False):
    N, CORES, OWN, WIN, NW = (cfg[k] for k in ("N", "CORES", "OWN", "WIN", "NW"))
    F_IN, H, D, OUT = cfg["F_IN"], cfg["H"], cfg["D"], cfg["OUT"]
    ROW0, ROW1, SPLIT = cfg["ROW0"], cfg["ROW1"], cfg["SPLIT"]
    NROWS = cfg["NROWS"]
    KCH = (F_IN + 127) // 128
    C0 = F_IN + H                 # 264
    W1C = OUT + 2 + OUT           # 130
    NTP0, NTP1 = meta["l0"]["ntp"], meta["l1"]["ntp"]

    nc = bacc.Bacc("TRN2", target_bir_lowering=False, debug=False,
                   num_devices=CORES)

    xTb = nc.dram_tensor("xTb", [F_IN, N], BF16, kind="ExternalInput")
    xTo = nc.dram_tensor("xTo", [F_IN, OWN], BF16, kind="ExternalInput")
    idx0_in = nc.dram_tensor("idx0", [128, meta["l0"]["idx_cols"]], dt.int16, kind="ExternalInput")
    idx1_in = nc.dram_tensor("idx1", [128, meta["l1"]["idx_cols"]], dt.int16, kind="ExternalInput")
    dstoh0_in = nc.dram_tensor("dstoh0", [128, meta["l0"]["t_tiles"] * 128], BF16, kind="ExternalInput")
    dstoh1_in = nc.dram_tensor("dstoh1", [128, meta["l1"]["t_tiles"] * 128], BF16, kind="ExternalInput")
    dstohT0_in = nc.dram_tensor("dstohT0", [128, meta["l0"]["t_tiles"] * 128], BF16, kind="ExternalInput")
    dstohT1_in = nc.dram_tensor("dstohT1", [128, meta["l1"]["t_tiles"] * 128], BF16, kind="ExternalInput")
    dstf0_in = nc.dram_tensor("dstf0", [1, meta["l0"]["t_tiles"] * 128], BF16, kind="ExternalInput")
    dstf1_in = nc.dram_tensor("dstf1", [1, meta["l1"]["t_tiles"] * 128], BF16, kind="ExternalInput")
    wcat0_in = nc.dram_tensor("wcat0b", [F_IN, C0], BF16, kind="ExternalInput")
    wad0_in = nc.dram_tensor("wad0b", [F_IN, H], BF16, kind="ExternalInput")
    wr0_in = nc.dram_tensor("wr0b", [F_IN, F_IN], BF16, kind="ExternalInput")
    wcat1_in = nc.dram_tensor("wcat1b", [F_IN, W1C], BF16, kind="ExternalInput")
    biasr1_in = nc.dram_tensor("biasr1b", [1, W1C], BF16, kind="ExternalInput")
    bias0_in = nc.dram_tensor("bias0", [128, KCH], F32, kind="ExternalInput")
    ident_in = nc.dram_tensor("identf", [128, 128], F32, kind="ExternalInput")
    identb_in = nc.dram_tensor("identb", [128, 128], BF16, kind="ExternalInput")
    iota_in = nc.dram_tensor("iotab", [128, 128], BF16, kind="ExternalInput")
    iotaP_in = nc.dram_tensor("iotaPf", [128, 1], F32, kind="ExternalInput")
    ones_in = nc.dram_tensor("onesb", [1, 128], BF16, kind="ExternalInput")
    out_own = nc.dram_tensor("out_own", [OWN, OUT], F32, kind="ExternalOutput")

    tbl1_shard = nc.dram_tensor("tbl1_shard", [OWN, ROW1], BF16)
    reg0 = nc.dram_tensor("reg0", [NROWS, ROW0], BF16)
    reg1 = nc.dram_tensor("reg1", [NROWS, ROW1], BF16,
                          addr_space="Shared" if CORES > 1 else "Local")
    res0T_d = nc.dram_tensor("res0T_d", [128, KCH, OWN], BF16)

    rg = [list(range(CORES))]

    def win_size(w):
        return min(WIN, OWN - w * WIN)

    ag_after_w = {}
    for (w0, w1, r0, r1) in cfg["CHUNKS"]:
        ag_after_w[w1 - 1] = (r0, r1, 1 + CORES * r0, 1 + CORES * r1)

    with tile.TileContext(nc) as tc, ExitStack() as ctx:
        const = ctx.enter_context(tc.tile_pool(name="const", bufs=1))
        persist = ctx.enter_context(tc.tile_pool(name="persist", bufs=1))

        wcat0_t = const.tile([128, KCH, C0], BF16)
        wad0_t = const.tile([128, KCH, H], BF16)
        wr0_t = const.tile([128, KCH, F_IN], BF16)
        wcat1_t = const.tile([128, KCH, W1C], BF16)
        for k in range(KCH):
            nc.sync.dma_start(wcat0_t[:, k, :], wcat0_in[k * 128:(k + 1) * 128, :])
            nc.sync.dma_start(wad0_t[:, k, :], wad0_in[k * 128:(k + 1) * 128, :])
            nc.sync.dma_start(wr0_t[:, k, :], wr0_in[k * 128:(k + 1) * 128, :])
            nc.sync.dma_start(wcat1_t[:, k, :], wcat1_in[k * 128:(k + 1) * 128, :])
        biasr1_t = const.tile([1, W1C], BF16)
        nc.sync.dma_start(biasr1_t[:], biasr1_in[:])
        bias0_t = const.tile([128, KCH], F32)
        nc.sync.dma_start(bias0_t[:], bias0_in[:])
        ident_t = const.tile([128, 128], F32)
        nc.sync.dma_start(ident_t[:], ident_in[:])
        identb_t = const.tile([128, 128], BF16)
        nc.sync.dma_start(identb_t[:], identb_in[:])
        iota_t = const.tile([128, 128], BF16)
        nc.sync.dma_start(iota_t[:], iota_in[:])
        iotaP_t = const.tile([128, 1], F32)
        nc.sync.dma_start(iotaP_t[:], iotaP_in[:])
        ones_t = const.tile([1, 128], BF16)
        nc.sync.dma_start(ones_t[:], ones_in[:])

        idx0_t = persist.tile([128, meta["l0"]["idx_cols"]], dt.int16)
        nc.sync.dma_start(idx0_t[:], idx0_in[:])
        idx1_t = persist.tile([128, meta["l1"]["idx_cols"]], dt.int16)
        nc.sync.dma_start(idx1_t[:], idx1_in[:])

        for _rep in range(repeat):
            al_dst0b = persist.tile([128, NW, H], BF16)
            al_dst1b = persist.tile([128, NW, 1], BF16)
            res1 = persist.tile([128, NW, OUT], F32)
            o1f = persist.tile([128, NW, OUT], F32)
            svs = persist.tile([128, NW], F32)

            # ---- sentinels --------------------------------------------------
            with tc.tile_pool(name="sent", bufs=1) as sp:
                s0 = sp.tile([1, ROW0], BF16)
                nc.vector.memset(s0[:], 0.0)
                nc.vector.memset(s0[:, F_IN:F_IN + H], -1e30)
                nc.sync.dma_start(reg0[0:1, :], s0[:])
                nc.sync.dma_start(reg0[NROWS - 1:NROWS, :], s0[:])
                s1 = sp.tile([1, ROW1], BF16)
                nc.vector.memset(s1[:], 0.0)
                nc.vector.memset(s1[:, OUT:OUT + 1], -1e30)
                nc.sync.dma_start(reg1[0:1, :], s1[:])
                nc.sync.dma_start(reg1[NROWS - 1:NROWS, :], s1[:])

            # ---- setup A: own-shard res0T + al_d0 ---------------------------
            CH = 1024
            with tc.tile_pool(name="oxp", bufs=2) as oxp, \
                 tc.tile_pool(name="ops", bufs=2, space="PSUM") as ops, \
                 tc.tile_pool(name="opal", bufs=2, space="PSUM") as opal, \
                 tc.tile_pool(name="osb", bufs=2) as osb:
                for j0 in range(0, OWN, CH):
                    j1 = min(j0 + CH, OWN)
                    jw = j1 - j0
                    xo = oxp.tile([128, KCH, CH], BF16, tag="xo")
                    for k in range(KCH):
                        nc.sync.dma_start(xo[:, k, :jw], xTo[k * 128:(k + 1) * 128, j0:j1])
                    rstage = osb.tile([128, KCH, CH], BF16, tag="rstage")
                    for fc in range(KCH):
                        for h0 in range(0, jw, 512):
                            hw_ = min(512, jw - h0)
                            ps_r = ops.tile([128, 512], F32, tag="ps_r")
                            for k in range(KCH):
                                nc.tensor.matmul(ps_r[:, :hw_],
                                                 wr0_t[:, k, fc * 128:(fc + 1) * 128],
                                                 xo[:, k, h0:h0 + hw_],
                                                 start=(k == 0), stop=(k == KCH - 1))
                            nc.scalar.activation(rstage[:, fc, h0:h0 + hw_],
                                                 ps_r[:, :hw_], ACT.Identity,
                                                 bias=bias0_t[:, fc:fc + 1])
                    nc.sync.dma_start(res0T_d[:, :, j0:j1], rstage[:, :, :jw])
                    # al_d0 for the 8 windows in this chunk
                    nwc = (jw + 127) // 128
                    ps_al = opal.tile([128, CH // 128, H], F32, tag="ps_al")
                    for jj in range(nwc):
                        cw = min(128, jw - jj * 128)
                        for k in range(KCH):
                            nc.tensor.matmul(ps_al[:cw, jj, :],
                                             xo[:, k, jj * 128:jj * 128 + cw],
                                             wad0_t[:, k, :],
                                             start=(k == 0), stop=(k == KCH - 1))
                    w0 = j0 // 128
                    nc.vector.tensor_copy(al_dst0b[:, w0:w0 + nwc, :],
                                          ps_al[:, :nwc, :])

            # ---- setup B: full-N table0 (redundant on every core) ----------
            with tc.tile_pool(name="bxp", bufs=2) as bxp, \
                 tc.tile_pool(name="bps", bufs=2, space="PSUM") as bps, \
                 tc.tile_pool(name="bsb", bufs=2) as bsb:
                for j0 in range(0, N, CH):
                    j1 = min(j0 + CH, N)
                    jw = j1 - j0
                    nfull = jw // 128          # full 128-row blocks
                    xs = bxp.tile([128, KCH, CH], BF16, tag="xs")
                    for k in range(KCH):
                        nc.sync.dma_start(xs[:, k, :jw], xTb[k * 128:(k + 1) * 128, j0:j1])
                    stage = bsb.tile([128, CH // 128, ROW0], BF16, tag="stage")
                    for g0 in range(0, jw, 512):
                        gn = min(4, (jw - g0 + 127) // 128)
                        ps_a = bps.tile([128, 4, 512], F32, tag="ps_a")
                        for q in range(gn):
                            jj = g0 + q * 128
                            cw = min(128, jw - jj)
                            for k in range(KCH):
                                nc.tensor.matmul(ps_a[:cw, q, :C0], xs[:, k, jj:jj + cw],
                                                 wcat0_t[:, k, :],
                                                 start=(k == 0), stop=(k == KCH - 1))
                        nc.vector.tensor_copy(stage[:, g0 // 128:g0 // 128 + gn, :C0],
                                              ps_a[:, :gn, :C0])
                    if nfull:
                        nc.sync.dma_start(
                            reg0[1 + j0:1 + j0 + nfull * 128, :]
                                .rearrange("(q p) e -> p q e", p=128),
                            stage[:, :nfull, :])
                    if nfull * 128 < jw:
                        cw = jw - nfull * 128
                        nc.sync.dma_start(
                            reg0[1 + j0 + nfull * 128:1 + j0 + jw, :],
                            stage[:cw, nfull, :])

            # ---- shared window loop -----------------------------------------
            def layer(lidx, region, elem, feat, nh, al_dst_t, idx_t, dstoh_t,
                      dstohT_t, dstf_t, lm, ntp):
                lo_slots, hi_slots = lm["lo_slots"], lm["hi_slots"]
                idx_off = 0
                tile_off = 0
                with tc.tile_pool(name=f"G{lidx}", bufs=gbufs) as gp, \
                     tc.tile_pool(name=f"w{lidx}", bufs=wbufs) as wp, \
                     tc.tile_pool(name=f"v{lidx}", bufs=vbufs) as vp, \
                     tc.tile_pool(name=f"agg{lidx}", bufs=2, space="PSUM") as pp, \
                     tc.tile_pool(name=f"rep{lidx}", bufs=2, space="PSUM") as pr, \
                     tc.tile_pool(name=f"ad{lidx}", bufs=2, space="PSUM") as pa, \
                     tc.tile_pool(name=f"cons{lidx}", bufs=1, space="PSUM") as pcons, \
                     tc.tile_pool(name=f"tail{lidx}", bufs=2) as tl:
                    for w in range(NW):
                        ws = win_size(w)
                        L, Hs = lo_slots[w], hi_slots[w]
                        nt = (L + Hs) // 128
                        G = gp.tile([128, ntp, elem], BF16, tag="G")
                        GCAP = 896
                        for base, nsl, ap_in in ((0, L, region[:, :]),
                                                 (L, Hs, region[SPLIT + 1:, :])):
                            for c0 in range(0, nsl, GCAP):
                                csl = min(GCAP, nsl - c0)
                                sb = base + c0
                                nc.gpsimd.dma_gather(
                                    out_ap=G[:, sb // 128:(sb + csl) // 128, :],
                                    in_ap=ap_in,
                                    idxs_ap=idx_t[:, idx_off + sb // 16:
                                                  idx_off + (sb + csl) // 16],
                                    num_idxs=csl, num_idxs_reg=csl, elem_size=elem)
                        idx_off += (L + Hs) // 16

                        if w2t_host:
                            W2T_w = gp.tile([128, ntp, 128], BF16, tag="W2Tw")
                            nc.sync.dma_start(
                                W2T_w[:, :nt, :].rearrange("p t e -> p (t e)"),
                                dstohT_t[:, tile_off * 128:(tile_off + nt) * 128])
                        else:
                            dsl = gp.tile([1, ntp * 128], BF16, tag="dsl")
                            nc.sync.dma_start(dsl[:, :nt * 128],
                                              dstf_t[0:1, tile_off * 128:
                                                     (tile_off + nt) * 128])
                        W2 = gp.tile([128, ntp, 128], BF16, tag="W2")
                        nc.sync.dma_start(
                            W2[:, :nt, :].rearrange("p t e -> p (t e)"),
                            dstoh_t[:, tile_off * 128:(tile_off + nt) * 128])
                        ps_agg = pp.tile([128, feat + nh], F32, tag="agg")
                        for b0 in range(0, nt, sub):
                            nb = min(sub, nt - b0)
                            if not w2t_host:
                                rep = pr.tile([128, sub * 128], F32, tag="rep")
                                nc.tensor.matmul(
                                    rep[:, :nb * 128], ones_t[:],
                                    dsl[0:1, b0 * 128:(b0 + nb) * 128],
                                    start=True, stop=True)
                                W2T = wp.tile([128, sub, 128], BF16, tag="W2T")
                                nc.vector.tensor_scalar(
                                    W2T[:, :nb, :].rearrange("p t e -> p (t e)"),
                                    rep[:, :nb * 128], iotaP_t[:], None, AL.is_equal)
                            ps_ad = pa.tile([128, sub, nh], F32, tag="ad")
                            for t in range(nb):
                                W2T_ap = (W2T_w[:ws, b0 + t, :] if w2t_host
                                          else W2T[:ws, t, :])
                                nc.tensor.matmul(ps_ad[:, t, :], W2T_ap,
                                                 al_dst_t[:ws, w, :],
                                                 start=True, stop=True)
                            s0b = vp.tile([128, sub, nh], BF16, tag="s0b")
                            nc.vector.tensor_copy(s0b[:, :nb, :], ps_ad[:, :nb, :])
                            s_t = vp.tile([128, sub, nh], BF16, tag="s")
                            nc.vector.tensor_tensor(s_t[:, :nb, :],
                                                    G[:, b0:b0 + nb, feat:feat + nh],
                                                    s0b[:, :nb, :], AL.add)
                            nc.vector.scalar_tensor_tensor(
                                s_t[:, :nb, :], s_t[:, :nb, :], NEG_SLOPE,
                                s_t[:, :nb, :], AL.mult, AL.max)
                            V = vp.tile([128, sub, feat + nh], BF16, tag="V")
                            nc.scalar.activation(V[:, :nb, feat:feat + nh],
                                                 s_t[:, :nb, :], ACT.Exp)
                            Gb = G[:, b0:b0 + nb, :]
                            if nh == 1:
                                nc.vector.tensor_tensor(
                                    V[:, :nb, :feat], Gb[:, :, :feat],
                                    V[:, :nb, feat:feat + nh]
                                        .broadcast_to((128, nb, feat)),
                                    AL.mult)
                            else:
                                nc.vector.tensor_tensor(
                                    V[:, :nb, :feat].rearrange(
                                        "p t (k d) -> p t k d", k=nh),
                                    Gb[:, :, :feat].rearrange(
                                        "p t (k d) -> p t k d", k=nh),
                                    V[:, :nb, feat:feat + nh].unsqueeze(3)
                                        .broadcast_to((128, nb, nh, D)),
                                    AL.mult)
                            for t in range(nb):
                                gt = b0 + t
                                nc.tensor.matmul(ps_agg[:], W2[:, gt, :],
                                                 V[:, t, :],
                                                 start=(gt == 0),
                                                 stop=(gt == nt - 1))
                        tile_off += nt

                        den = tl.tile([128, nh], F32, tag="den")
                        nc.vector.tensor_scalar(den[:ws, :], ps_agg[:ws, feat:],
                                                1e-16, None, AL.add)
                        rden = tl.tile([128, nh], F32, tag="rden")
                        nc.vector.reciprocal(rden[:ws, :], den[:ws, :])
                        o0 = tl.tile([128, feat], F32, tag="o0")
                        if nh == 1:
                            nc.vector.tensor_scalar(o0[:ws, :], ps_agg[:ws, :feat],
                                                    rden[:ws, :], None, AL.mult)
                        else:
                            nc.vector.tensor_tensor(
                                o0[:ws, :].rearrange("p (k d) -> p k d", k=nh),
                                ps_agg[:ws, :feat].rearrange("p (k d) -> p k d", k=nh),
                                rden[:ws, :].unsqueeze(2).broadcast_to((ws, nh, D)),
                                AL.mult)
                        yield w, ws, o0, tl, pcons

            # ---- layer 0 + consumer ----------------------------------------
            for w, ws, o0, tl, pcons in layer(
                    0, reg0, ROW0, F_IN, H, al_dst0b, idx0_t, dstoh0_in,
                    dstohT0_in, dstf0_in, meta["l0"], NTP0):
                r0 = w * WIN
                res_w = tl.tile([128, KCH, 128], BF16, tag="res_w")
                nc.sync.dma_start(res_w[:, :, :ws], res0T_d[:, :, r0:r0 + ws])
                res_f = tl.tile([128, KCH, 128], F32, tag="res_f")
                nc.vector.tensor_copy(res_f[:, :, :ws], res_w[:, :, :ws])
                tp = pcons.tile([128, KCH, 128], F32, tag="tp")
                for k in range(KCH):
                    nc.tensor.transpose(tp[:, k, :ws], o0[:ws, k * 128:(k + 1) * 128],
                                        ident_t[:ws, :ws])
                xr = tl.tile([128, KCH, 128], F32, tag="xr")
                nc.vector.tensor_tensor(xr[:, :, :ws], tp[:, :, :ws],
                                        res_f[:, :, :ws], AL.add)
                mn = tl.tile([128, KCH, 128], F32, tag="mn")
                nc.vector.tensor_scalar(mn[:, :, :ws], xr[:, :, :ws], 0.0, None, AL.min)
                nc.scalar.activation(mn[:, :, :ws], mn[:, :, :ws], ACT.Exp)
                # hT = max(xr,0) + exp(min(xr,0))  (== ELU+1; -1 folded in bias)
                hT = tl.tile([128, KCH, 128], BF16, tag="hT")
                nc.vector.scalar_tensor_tensor(hT[:, :, :ws], xr[:, :, :ws], 0.0,
                                               mn[:, :, :ws], AL.max, AL.add)
                ps_t1 = pcons.tile([128, W1C], F32, tag="t1")
                for k in range(KCH):
                    nc.tensor.matmul(ps_t1[:ws, :], hT[:, k, :ws], wcat1_t[:, k, :],
                                     start=(k == 0), stop=False)
                nc.tensor.matmul(ps_t1[:ws, :], ones_t[:, :ws], biasr1_t[:],
                                 start=False, stop=True)
                st1 = tl.tile([128, ROW1], BF16, tag="st1")
                nc.vector.tensor_copy(st1[:ws, :OUT + 1], ps_t1[:ws, :OUT + 1])
                nc.vector.tensor_copy(al_dst1b[:ws, w, :], ps_t1[:ws, OUT + 1:OUT + 2])
                nc.vector.tensor_copy(res1[:ws, w, :], ps_t1[:ws, OUT + 2:])
                nc.sync.dma_start(tbl1_shard[r0:r0 + ws, :], st1[:ws, :])

                if w in ag_after_w:
                    sr0, sr1, rr0, rr1 = ag_after_w[w]
                    if CORES == 1 or sim_no_collective:
                        nc.sync.dma_start(reg1[rr0:rr0 + (sr1 - sr0), :],
                                          tbl1_shard[sr0:sr1, :])
                    else:
                        nc.gpsimd.collective_compute(
                            "AllGather", AL.bypass, replica_groups=rg,
                            ins=[tbl1_shard[sr0:sr1, :].opt()],
                            outs=[reg1[rr0:rr1, :].opt()])

            # ---- layer 1 + consumer (log-softmax Ln batched at the end) ----
            for w, ws, o0, tl, pcons in layer(
                    1, reg1, ROW1, OUT, 1, al_dst1b, idx1_t, dstoh1_in,
                    dstohT1_in, dstf1_in, meta["l1"], NTP1):
                nc.vector.tensor_tensor(o0[:ws, :], o0[:ws, :], res1[:ws, w, :], AL.add)
                mxv = tl.tile([128, 1], F32, tag="mxv")
                nc.vector.reduce_max(mxv[:ws, :], o0[:ws, :], axis=mybir.AxisListType.X)
                nc.vector.tensor_scalar(o1f[:ws, w, :], o0[:ws, :], mxv[:ws, :], None,
                                        AL.subtract)
                ev = tl.tile([128, OUT], F32, tag="ev")
                nc.scalar.activation(ev[:ws, :], o1f[:ws, w, :], ACT.Exp)
                nc.vector.reduce_sum(svs[:ws, w:w + 1], ev[:ws, :],
                                     axis=mybir.AxisListType.X)

            with tc.tile_pool(name="fin", bufs=2) as fp:
                lnv = fp.tile([128, NW], F32)
                nc.scalar.activation(lnv[:], svs[:], ACT.Ln)
                for w in range(NW):
                    ws = win_size(w)
                    ot = fp.tile([128, OUT], F32, tag="ot")
                    nc.vector.tensor_scalar(ot[:ws, :], o1f[:ws, w, :],
                                            lnv[:ws, w:w + 1], None, AL.subtract)
                    nc.sync.dma_start(out_own[w * WIN:w * WIN + ws, :], ot[:ws, :])

    nc.compile()
    return nc


# ----------------------------------------------------------------- entrypoint

_CORES = 8
BUILD_KW = dict(w2t_host=False)
CFG_KW = dict(ag_chunks=1)


def kernel(x, edge_index, W0, a_src0, a_dst0, b0, Wr0, br0,
           W1, a_src1, a_dst1, b1, Wr1, br1):
    """Full-input GAT kernel: shards across 8 NeuronCores internally."""
    x = np.asarray(x)
    edge_index = np.asarray(edge_index)
    N, F_in = x.shape
    E = edge_index.shape[1]
    H, D = np.asarray(a_src0).shape
    OUT = np.asarray(a_src1).shape[1]
    cfg = make_cfg(N, E, F_in, H, D, OUT, _CORES, **CFG_KW)
    weights = dict(
        W0=np.asarray(W0, np.float32), a_src0=np.asarray(a_src0, np.float32),
        a_dst0=np.asarray(a_dst0, np.float32), b0=np.asarray(b0, np.float32),
        Wr0=np.asarray(Wr0, np.float32), br0=np.asarray(br0, np.float32),
        W1=np.asarray(W1, np.float32), a_src1=np.asarray(a_src1, np.float32),
        a_dst1=np.asarray(a_dst1, np.float32), b1=np.asarray(b1, np.float32),
        Wr1=np.asarray(Wr1, np.float32), br1=np.asarray(br1, np.float32))
    in_maps, meta = prepare(x.astype(np.float32), edge_index, weights, cfg)
    nc = build(cfg, meta, **BUILD_KW)
    from concourse.bass_utils import run_bass_kernel_spmd
    res = run_bass_kernel_spmd(nc, in_maps, list(range(_CORES)))
    out = np.concatenate([res.results[c]["out_own"] for c in range(_CORES)],
                         axis=0).astype(np.float32)
    return out
